# revision 1
# baseline (speedup 1.0000x reference)
"""AttentionPool2d kernel for 8 Trainium2 NeuronCores.

Only the CLS-token output of the attention is returned by the reference, so
the N x N attention collapses to single-query attention per (batch, head):

  t' = [x tokens + pos_emb[1:]]  (1024 tokens), CLS = mean(x) + pos_emb[0]
  q      = CLS @ (Wq*scale) + bq*scale                       [256]
  w_s    = sum_k Wk[d,h,k] * q[h*32+k]                       [256, 8]
  scores = t' @ w_s      (bk shifts all logits equally -> softmax-invariant)
  attn   = softmax over 1025 tokens
  u[h]   = sum_m attn[h,m] t'[m]                             [8, 256]
  out    = sum_h u[h] @ (Wv[:,h,:] @ Wo[h]) + (bo + sum_h bv[h] @ Wo[h])

Sharding: data-parallel over batch, 8 batches per core.
"""

import sys

sys.path.insert(0, "/opt/trn_rl_repo")

from contextlib import ExitStack

import numpy as np

import concourse.bacc as bacc
import concourse.bass as bass  # noqa: F401
import concourse.tile as tile
from concourse import mybir
from concourse.bass_utils import run_bass_kernel_spmd

F32 = mybir.dt.float32
AF = mybir.ActivationFunctionType
ALU = mybir.AluOpType

B, D, H, DK, O = 64, 256, 8, 32, 256
NT = 1024          # non-CLS tokens
BPC = B // 8       # batches per core
NI = NT // 128     # token tiles per batch


def build_program():
    nc = bacc.Bacc(
        "TRN2",
        target_bir_lowering=False,
        debug=False,
        enable_asserts=False,
        num_devices=8,
    )
    xs = nc.dram_tensor("xs", [BPC, NT, D], F32, kind="ExternalInput").ap()
    posB = nc.dram_tensor("posB", [128, NI * D], F32, kind="ExternalInput").ap()
    wq = nc.dram_tensor("wq", [128, 2 * D], F32, kind="ExternalInput").ap()
    wk = nc.dram_tensor("wk", [128, 2 * D], F32, kind="ExternalInput").ap()
    bq = nc.dram_tensor("bq", [1, D], F32, kind="ExternalInput").ap()
    cadj = nc.dram_tensor("cadj", [128, 2], F32, kind="ExternalInput").ap()
    wvo = nc.dram_tensor("wvo", [128, 16 * O], F32, kind="ExternalInput").ap()
    bout = nc.dram_tensor("bout", [BPC, O], F32, kind="ExternalInput").ap()
    ident = nc.dram_tensor("ident", [128, 128], F32, kind="ExternalInput").ap()
    ones1 = nc.dram_tensor("ones1", [1, 128], F32, kind="ExternalInput").ap()
    out_d = nc.dram_tensor("out", [BPC, O], F32, kind="ExternalOutput").ap()

    xr = xs.rearrange("b (i p) d -> b p i d", p=128)

    with tile.TileContext(nc) as tc, ExitStack() as ctx:
        wpool = ctx.enter_context(tc.tile_pool(name="weights", bufs=1))
        xpool = ctx.enter_context(tc.tile_pool(name="x", bufs=4))
        tpool = ctx.enter_context(tc.tile_pool(name="t", bufs=4))
        apool = ctx.enter_context(tc.tile_pool(name="tA", bufs=4))
        spool = ctx.enter_context(tc.tile_pool(name="smalls", bufs=3))
        epool = ctx.enter_context(tc.tile_pool(name="escore", bufs=4))
        # PSUM: 8 banks total
        tr_ps = ctx.enter_context(tc.tile_pool(name="trps", bufs=3, space="PSUM"))
        sc_ps = ctx.enter_context(tc.tile_pool(name="scps", bufs=2, space="PSUM"))
        sm_ps = ctx.enter_context(tc.tile_pool(name="smps", bufs=1, space="PSUM"))
        uT_ps = ctx.enter_context(tc.tile_pool(name="utps", bufs=1, space="PSUM"))

        posB_s = wpool.tile([128, NI * D], F32, tag="posB")
        nc.sync.dma_start(posB_s[:], posB)
        wq_s = wpool.tile([128, 2 * D], F32, tag="wq")
        nc.sync.dma_start(wq_s[:], wq)
        wk_s = wpool.tile([128, 2 * D], F32, tag="wk")
        nc.sync.dma_start(wk_s[:], wk)
        bq_s = wpool.tile([1, D], F32, tag="bq")
        nc.sync.dma_start(bq_s[:], bq)
        cadj_s = wpool.tile([128, 2], F32, tag="cadj")
        nc.sync.dma_start(cadj_s[:], cadj)
        id_s = wpool.tile([128, 128], F32, tag="ident")
        nc.sync.dma_start(id_s[:], ident)
        on_s = wpool.tile([1, 128], F32, tag="ones1")
        nc.sync.dma_start(on_s[:], ones1)
        uT_all = wpool.tile([128, 128], F32, tag="uTall")  # (c,b,h) cols

        state = {}

        def stage_a(b):
                # 1. load x[b] -> [128 tok-part, (i,d)] in two half-DMAs
                xB = xpool.tile([128, NI * D], F32, tag="xB")
                half = NI // 2 * D
                for g in range(2):
                    nc.sync.dma_start(
                        xB[:, g * half : (g + 1) * half].rearrange(
                            "p (i d) -> p i d", d=D
                        ),
                        xr[b][:, g * (NI // 2) : (g + 1) * (NI // 2)],
                    )
                # 2. add pos per chunk -> t' tokens 1..1024, layout B
                tB = tpool.tile([128, NI * D], F32, tag="tB")
                for i in range(NI):
                    eng = nc.vector if i % 2 == 0 else nc.gpsimd
                    eng.tensor_tensor(
                        tB[:, i * D : (i + 1) * D],
                        xB[:, i * D : (i + 1) * D],
                        posB_s[:, i * D : (i + 1) * D],
                        op=ALU.add,
                    )
                # 3. PE-transpose to layout A: tA[:, c, m] = t'[m, c*128+p]
                # 4 transposes share one PSUM bank -> 1 big copy out
                tA = apool.tile([128, 2, NT + 8], F32, tag="tA")
                for g in range(4):
                    tr = tr_ps.tile([128, 512], F32, tag="tr", name=f"tr_{b}_{g}")
                    for j in range(4):
                        i, c = (g * 4 + j) // 2, (g * 4 + j) % 2
                        nc.tensor.transpose(
                            tr[:, j * 128 : (j + 1) * 128],
                            tB[:, i * D + c * 128 : i * D + (c + 1) * 128],
                            id_s[:],
                        )
                    cp = nc.scalar.copy if g % 2 == 0 else nc.vector.tensor_copy
                    cp(
                        tA[:, :, 2 * g * 128 : 2 * g * 128 + 256].rearrange(
                            "p c (il m) -> p c il m", m=128
                        ),
                        tr[:].rearrange("p (il c m) -> p c il m", c=2, m=128),
                    )
                # 4. CLS column: mean over tokens + cls_adj -> tA[:, c, 1024]
                # partial sums per transpose-copy group so the mean chain
                # starts before the last copy lands
                for c in range(2):
                    parts = spool.tile([128, 4], F32, tag="parts")
                    for g in range(4):
                        nc.vector.reduce_sum(
                            out=parts[:, g : g + 1],
                            in_=tA[:, c, g * 256 : (g + 1) * 256],
                            axis=mybir.AxisListType.X,
                        )
                    tsum = spool.tile([128, 1], F32, tag="tsum")
                    nc.vector.reduce_sum(
                        out=tsum[:], in_=parts[:], axis=mybir.AxisListType.X
                    )
                    nc.vector.tensor_scalar(
                        out=tA[:, c, NT : NT + 1],
                        in0=tsum[:],
                        scalar1=1.0 / NT,
                        scalar2=cadj_s[:, c : c + 1],
                        op0=ALU.mult,
                        op1=ALU.add,
                    )
                # 5. q = CLS @ Wq*scale + bq*scale
                q_ps = sm_ps.tile([1, D], F32, tag="smq")
                for c in range(2):
                    nc.tensor.matmul(
                        q_ps[:],
                        tA[:, c, NT : NT + 1],
                        wq_s[:, c * D : (c + 1) * D],
                        start=(c == 0),
                        stop=(c == 1),
                    )
                q_sb = spool.tile([1, D], F32, tag="qsb")
                nc.vector.tensor_tensor(q_sb[:], q_ps[:], bq_s[:], op=ALU.add)
                # broadcast q across 128 partitions via rank-1 matmul
                qbc_ps = sm_ps.tile([128, D], F32, tag="smq", name=f"qbc_{b}")
                nc.tensor.matmul(qbc_ps[:], on_s[:], q_sb[:], start=True, stop=True)
                # 6. w_s[d, h] = sum_k Wk[d, h*32+k] q[h*32+k]
                w_s = spool.tile([128, 2 * H], F32, tag="ws")
                for c in range(2):
                    wtmp = epool.tile([128, D], F32, tag="wtmp")
                    nc.vector.tensor_tensor(
                        wtmp[:], wk_s[:, c * D : (c + 1) * D], qbc_ps[:], op=ALU.mult
                    )
                    nc.vector.reduce_sum(
                        out=w_s[:, c * H : (c + 1) * H],
                        in_=wtmp[:].rearrange("p (h k) -> p h k", k=DK),
                        axis=mybir.AxisListType.X,
                    )
                # 7. scores[h, m] = sum_d w_s[d, h] tA[d, m]
                scsb = epool.tile([H, NT + 8], F32, tag="scsb")
                for lo, n in ((0, 512), (512, 512), (NT, 1)):
                    ps = sc_ps.tile([H, 512], F32, tag="scps")
                    for c in range(2):
                        nc.tensor.matmul(
                            ps[:, 0:n],
                            w_s[:, c * H : (c + 1) * H],
                            tA[:, c, lo : lo + n],
                            start=(c == 0),
                            stop=(c == 1),
                        )
                    nc.vector.tensor_copy(scsb[:, lo : lo + n], ps[:, 0:n])
                state[b] = (tB, tA, scsb)

        def stage_b(b):
                tB, tA, scsb = state.pop(b)
                # 8. softmax (unnormalized exp; fold 1/Z later)
                nmx = spool.tile([H, 1], F32, tag="nmx")
                nc.vector.reduce_max(
                    out=nmx[:], in_=scsb[:, 0 : NT + 1], axis=mybir.AxisListType.X,
                    negate=True,
                )
                e_sb = epool.tile([H, NT + 8], F32, tag="esb")
                zs = spool.tile([H, 1], F32, tag="zs")
                nc.scalar.activation(
                    e_sb[:, 0 : NT + 1],
                    scsb[:, 0 : NT + 1],
                    AF.Exp,
                    bias=nmx[:],
                    scale=1.0,
                    accum_out=zs[:],
                )
                rz = spool.tile([H, 1], F32, tag="rz")
                nc.vector.reciprocal(rz[:], zs[:])
                # normalize per 128-token chunk so each eT transpose can
                # start as soon as its chunk is scaled
                for i in range(NI):
                    nc.vector.tensor_scalar(
                        out=e_sb[:, i * 128 : (i + 1) * 128],
                        in0=e_sb[:, i * 128 : (i + 1) * 128],
                        scalar1=rz[:], scalar2=None, op0=ALU.mult,
                    )
                nc.vector.tensor_scalar(
                    out=e_sb[:, NT : NT + 1], in0=e_sb[:, NT : NT + 1],
                    scalar1=rz[:], scalar2=None, op0=ALU.mult,
                )
                # 9. uT[c][d, h] = sum_m t'[m, d] attn[h, m]
                uT = [
                    uT_ps.tile([128, H], F32, tag=f"uT{c}", name=f"uT{c}_{b}")
                    for c in range(2)
                ]
                for i in range(NI):
                    etr = tr_ps.tile([128, H], F32, tag="tr")
                    nc.tensor.transpose(
                        etr[:], e_sb[0:H, i * 128 : (i + 1) * 128], id_s[0:H, 0:H]
                    )
                    eTs = spool.tile([128, H], F32, tag="eTs")
                    nc.vector.tensor_copy(eTs[:], etr[:])
                    for c in range(2):
                        nc.tensor.matmul(
                            uT[c][:],
                            tB[:, i * D + c * 128 : i * D + (c + 1) * 128],
                            eTs[:],
                            start=(i == 0),
                            stop=False,
                            skip_group_check=True,
                        )
                # CLS contribution: uT[c] += t0[c*128:...] outer attn_cls
                ecr = tr_ps.tile([1, H], F32, tag="tr")
                nc.tensor.transpose(ecr[:], e_sb[0:H, NT : NT + 1], id_s[0:H, 0:H])
                ecs = spool.tile([1, H], F32, tag="ecs")
                nc.vector.tensor_copy(ecs[:], ecr[:])
                t0r_sb = spool.tile([1, D], F32, tag="t0r")
                for c in range(2):
                    t0r = tr_ps.tile([1, 128], F32, tag="tr")
                    nc.tensor.transpose(t0r[:], tA[:, c, NT : NT + 1], id_s[:])
                    nc.vector.tensor_copy(t0r_sb[:, c * 128 : (c + 1) * 128], t0r[:])
                for c in range(2):
                    nc.tensor.matmul(
                        uT[c][:],
                        t0r_sb[:, c * 128 : (c + 1) * 128],
                        ecs[:],
                        start=False,
                        stop=True,
                        skip_group_check=True,
                    )
                    nc.vector.tensor_copy(
                        uT_all[:, c * 64 + b * H : c * 64 + (b + 1) * H], uT[c][:]
                    )

        PIPE = 3
        for b in range(PIPE):
            stage_a(b)
        # final-projection weights: DMA after the prologue so they don't
        # block the batch-0..2 x loads in the HWDGE FIFO
        wvo_s = wpool.tile([128, 16 * O], F32, tag="wvo")
        nc.sync.dma_start(wvo_s[:], wvo)
        bout_s = wpool.tile([BPC, O], F32, tag="bout")
        nc.sync.dma_start(bout_s[:], bout)
        for b in range(PIPE, BPC):
            stage_a(b)
            stage_b(b - PIPE)
        for b in range(BPC - PIPE, BPC):
            stage_b(b)
        # 10. out[b, o] = sum_{c,h} uT_all[:, c,b,h].T @ Wvo[c,h] + bout
        uv = uT_all[:].rearrange("p (c b h) -> p c b h", c=2, b=BPC)
        o_ps = sc_ps.tile([BPC, O], F32, tag="scps")
        for c in range(2):
            for h in range(H):
                nc.tensor.matmul(
                    o_ps[:],
                    uv[:, c, :, h],
                    wvo_s[:, (c * H + h) * O : (c * H + h + 1) * O],
                    start=(c == 0 and h == 0),
                    stop=(c == 1 and h == H - 1),
                )
        o_sb = epool.tile([BPC, O], F32, tag="osb")
        nc.vector.tensor_tensor(o_sb[:], o_ps[:], bout_s[:], op=ALU.add)
        nc.sync.dma_start(out_d, o_sb[:])
    nc.compile()
    return nc


def host_inputs(x, pos_emb, Wq, bq, Wk, bk, Wv, bv, Wo, bo):
    """Host-side weight preprocessing shared by all cores."""
    scale = np.float32(1.0 / np.sqrt(DK))
    pos_rest = pos_emb[1:]
    wq2 = (Wq.reshape(D, D) * scale).astype(np.float32)
    wk2 = Wk.reshape(D, H * DK).astype(np.float32)
    wvo = np.einsum("dhk,hko->hdo", Wv, Wo).astype(np.float32)
    bout = (bo + np.einsum("hk,hko->o", bv, Wo)).astype(np.float32)
    cls_adj = (pos_emb[0] - pos_rest.mean(0)).astype(np.float32)
    return {
        "posB": np.ascontiguousarray(
            pos_rest.reshape(NI, 128, D).transpose(1, 0, 2).reshape(128, NI * D)
        ),
        "wq": np.ascontiguousarray(np.concatenate([wq2[:128], wq2[128:]], axis=1)),
        "wk": np.ascontiguousarray(np.concatenate([wk2[:128], wk2[128:]], axis=1)),
        "bq": (bq.reshape(1, D) * scale).astype(np.float32),
        "cadj": np.ascontiguousarray(cls_adj.reshape(2, 128).T),
        "wvo": np.ascontiguousarray(
            np.concatenate(
                [wvo[h, c * 128 : (c + 1) * 128, :] for c in range(2) for h in range(H)],
                axis=1,
            )
        ),
        "bout": np.tile(bout.reshape(1, O), (BPC, 1)),
        "ident": np.eye(128, dtype=np.float32),
        "ones1": np.ones((1, 128), np.float32),
    }


_NC_CACHE = []


def _get_nc():
    if not _NC_CACHE:
        _NC_CACHE.append(build_program())
    return _NC_CACHE[0]


def run(trace=False, **inputs):
    nc = _get_nc()
    shared = host_inputs(**{k: np.asarray(v, np.float32) for k, v in inputs.items()})
    x = np.asarray(inputs["x"], np.float32).reshape(B, NT, D)
    in_maps = [
        dict(shared, xs=np.ascontiguousarray(x[j * BPC : (j + 1) * BPC]))
        for j in range(8)
    ]
    res = run_bass_kernel_spmd(nc, in_maps, core_ids=list(range(8)), trace=trace)
    out = np.concatenate([r["out"] for r in res.results], axis=0)
    return out, res


def kernel(**inputs):
    return run(trace=False, **inputs)[0]



# revision 5
# speedup vs baseline: 1.5989x; 1.5989x over previous
"""AttentionPool2d kernel for 8 Trainium2 NeuronCores.

Only the CLS-token output of the attention is returned by the reference, so
the N x N attention collapses to single-query attention per (batch, head):

  t'_m  = x_m + pos_emb[1+m]  (1024 tokens);  t_cls = mean_m(t'_m) + cadj
          with cadj = pos_emb[0] - mean(pos_emb[1:])
  q     = t_cls @ (Wq*scale) + bq*scale                     [256]
  w_s   = sum_k Wk[d,h,k] * q[h*32+k]                       [256, 8]
  scores= t' @ w_s   (bk shifts all logits equally -> softmax-invariant)
  attn  = softmax over 1025 tokens (1024 + CLS)
  u[h]  = sum_m attn[h,m] t'_m + attn_cls*(mean t' + cadj)
        = sum_m (attn[h,m]+attn_cls/1024) t'_m + attn_cls*cadj
  out   = sum_h u[h] @ (Wv[:,h,:] @ Wo[h]) + (bo + sum_h bv[h] @ Wo[h])

Wall time in this environment is dominated by host->device transfer over the
axon tunnel (~50-80 MB/s), so everything large is shipped as float16 and all
weights are packed into a single blob tensor.

Sharding: data-parallel over batch, 8 batches per core.
"""

import sys

sys.path.insert(0, "/opt/trn_rl_repo")

from contextlib import ExitStack

import numpy as np

import concourse.bacc as bacc
import concourse.bass as bass  # noqa: F401
import concourse.tile as tile
from concourse import mybir
from concourse.bass_utils import run_bass_kernel_spmd

F32 = mybir.dt.float32
F16 = mybir.dt.float16
AF = mybir.ActivationFunctionType
ALU = mybir.AluOpType

B, D, H, DK, O = 64, 256, 8, 32, 256
NT = 1024          # non-CLS tokens
BPC = B // 8       # batches per core
NI = NT // 128     # token tiles per batch

# blob column offsets (all f16, [128, C])
POS = 0            # posB [128, 2048], token-part layout
WQ = POS + NI * D          # 2048: wq2*scale, two 128-row halves side by side
WK = WQ + 2 * D            # 2560
WVO = WK + 2 * D           # 3072: 16 chunks [128, 256], chunk (c*H+h)
ID8 = WVO + 16 * O         # 7168: eye(8) on rows 0-7
CADJ = ID8 + 8             # 7176: cls_adj column form [128, 2]
CADJT = CADJ + 2           # 7178: cls_adj row form [1, 256] on row 0
BQ = CADJT + D             # 7434: bq*scale row form [1, 256] on row 0
BOUT = BQ + D              # 7690: bout [8, 256] on rows 0-7
C = BOUT + O               # 7946


def build_program():
    nc = bacc.Bacc(
        "TRN2",
        target_bir_lowering=False,
        debug=False,
        enable_asserts=False,
        num_devices=8,
    )
    xs = nc.dram_tensor("xs", [BPC, NT, D], F16, kind="ExternalInput").ap()
    blob = nc.dram_tensor("blob", [128, C], F16, kind="ExternalInput").ap()
    out_d = nc.dram_tensor("out", [BPC, O], F32, kind="ExternalOutput").ap()

    xr = xs.rearrange("b (i p) d -> b p i d", p=128)

    with tile.TileContext(nc) as tc, ExitStack() as ctx:
        wpool = ctx.enter_context(tc.tile_pool(name="weights", bufs=1))
        xpool = ctx.enter_context(tc.tile_pool(name="x", bufs=3))
        tpool = ctx.enter_context(tc.tile_pool(name="xT", bufs=3))
        spool = ctx.enter_context(tc.tile_pool(name="smalls", bufs=4))
        epool = ctx.enter_context(tc.tile_pool(name="esb", bufs=2))
        etpool = ctx.enter_context(tc.tile_pool(name="eT", bufs=2))
        scpool = ctx.enter_context(tc.tile_pool(name="scsb", bufs=2))
        # PSUM: 8 banks total -> 2 + 2 + 1 + 1 + 1 = 7
        q_ps = ctx.enter_context(tc.tile_pool(name="qps", bufs=2, space="PSUM"))
        sc_ps = ctx.enter_context(tc.tile_pool(name="scps", bufs=2, space="PSUM"))
        uT_ps = ctx.enter_context(tc.tile_pool(name="utps", bufs=1, space="PSUM"))
        tr_ps = ctx.enter_context(tc.tile_pool(name="trps", bufs=1, space="PSUM"))

        blob_s = wpool.tile([128, C], F16, tag="blob")
        nc.sync.dma_start(blob_s[:], blob)
        cadj32 = wpool.tile([128, 2], F32, tag="cadj32")
        nc.vector.tensor_copy(cadj32[:], blob_s[:, CADJ : CADJ + 2])
        ones16 = wpool.tile([1, 128], F16, tag="ones16")
        nc.vector.memset(ones16[:], 1.0)
        # posT[d, c, m]: transposed positional embedding, from 16 SBUF xbar
        # transposes of posB's [128, 128] sub-tiles (one-time cost)
        posT = wpool.tile([128, 2, NT], F16, tag="posT")
        for i in range(NI):
            for c in range(2):
                nc.sync.dma_start(
                    posT[:, c, i * 128 : (i + 1) * 128],
                    blob_s[:, POS + i * D + c * 128 : POS + i * D + (c + 1) * 128],
                    transpose=True,
                )
        uT_all = wpool.tile([128, 128], F16, tag="uTall")  # (c,b,h) cols

        for b in range(BPC):
            # 1. load x[b] in both layouts: token-major xB and d-major xT
            xB = xpool.tile([128, NI * D], F16, tag="xB")
            nc.sync.dma_start(
                xB[:].rearrange("p (i d) -> p i d", d=D), xr[b]
            )
            xT = tpool.tile([128, 2, NT], F16, tag="xT")
            for c in range(2):
                nc.sync.dma_start(
                    xT[:, c], xs[b][:, c * 128 : (c + 1) * 128], transpose=True
                )
            # 2. t' = x + pos, in place, both layouts
            nc.vector.tensor_tensor(xT[:, 0], xT[:, 0], posT[:, 0], op=ALU.add)
            nc.gpsimd.tensor_tensor(xT[:, 1], xT[:, 1], posT[:, 1], op=ALU.add)
            nc.vector.tensor_tensor(
                xB[:, 0:NT], xB[:, 0:NT], blob_s[:, POS : POS + NT], op=ALU.add
            )
            nc.gpsimd.tensor_tensor(
                xB[:, NT : 2 * NT],
                xB[:, NT : 2 * NT],
                blob_s[:, POS + NT : POS + 2 * NT],
                op=ALU.add,
            )
            # 3. CLS token: mean over tokens + cls_adj
            sums = spool.tile([128, 2], F32, tag="sums")
            t_cls = spool.tile([128, 2], F16, tag="tcls")
            for c in range(2):
                nc.vector.reduce_sum(
                    out=sums[:, c : c + 1], in_=xT[:, c], axis=mybir.AxisListType.X
                )
                nc.vector.tensor_scalar(
                    out=t_cls[:, c : c + 1],
                    in0=sums[:, c : c + 1],
                    scalar1=1.0 / NT,
                    scalar2=cadj32[:, c : c + 1],
                    op0=ALU.mult,
                    op1=ALU.add,
                )
            # 4. q = t_cls @ Wq*scale + bq*scale, broadcast to 128 partitions
            qp = q_ps.tile([1, D], F32, tag="q", name=f"q_{b}")
            for c in range(2):
                nc.tensor.matmul(
                    qp[:],
                    t_cls[:, c : c + 1],
                    blob_s[:, WQ + c * D : WQ + (c + 1) * D],
                    start=(c == 0),
                    stop=(c == 1),
                )
            q_sb = spool.tile([1, D], F16, tag="qsb")
            nc.vector.tensor_tensor(
                q_sb[:], qp[:], blob_s[0:1, BQ : BQ + D], op=ALU.add
            )
            qbc = q_ps.tile([128, D], F32, tag="q", name=f"qbc_{b}")
            nc.tensor.matmul(qbc[:], ones16[:], q_sb[:], start=True, stop=True)
            # 5. w_s[d, h] = sum_k Wk[d, h*32+k] q[h*32+k]
            w_s = spool.tile([128, 2 * H], F16, tag="ws")
            for c in range(2):
                wtmp = spool.tile([128, D], F16, tag="wtmp")
                nc.vector.tensor_tensor(
                    wtmp[:], blob_s[:, WK + c * D : WK + (c + 1) * D], qbc[:],
                    op=ALU.mult,
                )
                with nc.allow_low_precision(reason="w_s stored f16 for the PE"):
                    nc.vector.reduce_sum(
                        out=w_s[:, c * H : (c + 1) * H],
                        in_=wtmp[:].rearrange("p (h k) -> p h k", k=DK),
                        axis=mybir.AxisListType.X,
                    )
            # 6. scores[h, m] = sum_d w_s[d, h] t'[d, m]
            scsb = scpool.tile([H, NT + 32], F32, tag="scsb")
            for lo in (0, 512):
                ps = sc_ps.tile([H, 512], F32, tag="sc", name=f"sc_{b}_{lo}")
                for c in range(2):
                    nc.tensor.matmul(
                        ps[:],
                        w_s[:, c * H : (c + 1) * H],
                        xT[:, c, lo : lo + 512],
                        start=(c == 0),
                        stop=(c == 1),
                    )
                cp = nc.scalar.copy if lo == 0 else nc.vector.tensor_copy
                cp(scsb[:, lo : lo + 512], ps[:])
            ps = sc_ps.tile([H, 512], F32, tag="sc", name=f"sc_{b}_cls")
            for c in range(2):
                nc.tensor.matmul(
                    ps[:, 0:1],
                    w_s[:, c * H : (c + 1) * H],
                    t_cls[:, c : c + 1],
                    start=(c == 0),
                    stop=(c == 1),
                )
            nc.vector.tensor_copy(scsb[:, NT : NT + 1], ps[:, 0:1])
            # 7. softmax over 1025 logits; attn in f16 for the xbar transpose
            nmx = spool.tile([H, 1], F32, tag="nmx")
            nc.vector.reduce_max(
                out=nmx[:], in_=scsb[:, 0 : NT + 1], axis=mybir.AxisListType.X,
                negate=True,
            )
            e_sb = epool.tile([32, NT + 32], F16, tag="esb")
            nc.gpsimd.memset(e_sb[0:32, 0:NT], 0.0)
            zs = spool.tile([H, 1], F32, tag="zs")
            nc.scalar.activation(
                e_sb[0:H, 0 : NT + 1],
                scsb[:, 0 : NT + 1],
                AF.Exp,
                bias=nmx[:],
                scale=1.0,
                accum_out=zs[:],
            )
            rz = spool.tile([H, 1], F32, tag="rz")
            nc.vector.reciprocal(rz[:], zs[:])
            nc.vector.tensor_scalar(
                out=e_sb[0:H, 0 : NT + 1], in0=e_sb[0:H, 0 : NT + 1],
                scalar1=rz[:], scalar2=None, op0=ALU.mult,
            )
            # fold the CLS self-attention back onto the token weights:
            # a'_m = a_m + a_cls/1024  (u += a_cls * mean t')
            acl = spool.tile([H, 1], F32, tag="acl")
            nc.vector.tensor_scalar(
                out=acl[:], in0=e_sb[0:H, NT : NT + 1],
                scalar1=1.0 / NT, scalar2=None, op0=ALU.mult,
            )
            nc.vector.tensor_scalar(
                out=e_sb[0:H, 0:NT], in0=e_sb[0:H, 0:NT],
                scalar1=acl[:], scalar2=None, op0=ALU.add,
            )
            # 8. uT[c][d, h] = sum_m t'[m, d] a'[h, m] + cadj[d] a_cls[h]
            eT = etpool.tile([128, NI, 32], F16, tag="eT")
            nc.sync.dma_start(eT[:], e_sb[:, 0:NT], transpose=True)
            uT = [
                uT_ps.tile([128, H], F32, tag=f"uT{c}", name=f"uT{c}_{b}")
                for c in range(2)
            ]
            for i in range(NI):
                for c in range(2):
                    nc.tensor.matmul(
                        uT[c][:],
                        xB[:, i * D + c * 128 : i * D + (c + 1) * 128],
                        eT[:, i, 0:H],
                        start=(i == 0),
                        stop=False,
                        skip_group_check=True,
                    )
            ecr = tr_ps.tile([1, H], F16, tag="tr", name=f"ecr_{b}")
            nc.tensor.transpose(
                ecr[:], e_sb[0:H, NT : NT + 1], blob_s[0:H, ID8 : ID8 + 8]
            )
            ecs = spool.tile([1, H], F16, tag="ecs")
            nc.vector.tensor_copy(ecs[:], ecr[:])
            for c in range(2):
                nc.tensor.matmul(
                    uT[c][:],
                    blob_s[0:1, CADJT + c * 128 : CADJT + (c + 1) * 128],
                    ecs[:],
                    start=False,
                    stop=True,
                    skip_group_check=True,
                )
                nc.vector.tensor_copy(
                    uT_all[:, c * 64 + b * H : c * 64 + (b + 1) * H], uT[c][:]
                )
        # 9. out[b, o] = sum_{c,h} uT_all[:, c,b,h].T @ Wvo[c,h] + bout
        uv = uT_all[:].rearrange("p (c b h) -> p c b h", c=2, b=BPC)
        o_ps = sc_ps.tile([BPC, O], F32, tag="sc", name="o_ps")
        for c in range(2):
            for h in range(H):
                nc.tensor.matmul(
                    o_ps[:],
                    uv[:, c, :, h],
                    blob_s[:, WVO + (c * H + h) * O : WVO + (c * H + h + 1) * O],
                    start=(c == 0 and h == 0),
                    stop=(c == 1 and h == H - 1),
                )
        o_sb = spool.tile([BPC, O], F32, tag="osb")
        nc.vector.tensor_tensor(
            o_sb[:], o_ps[:], blob_s[0:BPC, BOUT : BOUT + O], op=ALU.add
        )
        nc.sync.dma_start(out_d, o_sb[:])
    nc.compile()
    return nc


def host_inputs(x, pos_emb, Wq, bq, Wk, bk, Wv, bv, Wo, bo):
    """Host-side weight preprocessing shared by all cores (packed f16 blob)."""
    f16 = np.float16
    scale = np.float32(1.0 / np.sqrt(DK))
    pos_rest = pos_emb[1:]
    wq2 = Wq.reshape(D, D) * scale
    wk2 = Wk.reshape(D, H * DK)
    wvo = np.einsum("dhk,hko->hdo", Wv, Wo)
    bout = bo + np.einsum("hk,hko->o", bv, Wo)
    cls_adj = pos_emb[0] - pos_rest.mean(0)

    blob = np.zeros((128, C), f16)
    blob[:, POS : POS + NI * D] = (
        pos_rest.reshape(NI, 128, D).transpose(1, 0, 2).reshape(128, NI * D)
    )
    blob[:, WQ : WQ + 2 * D] = np.concatenate([wq2[:128], wq2[128:]], axis=1)
    blob[:, WK : WK + 2 * D] = np.concatenate([wk2[:128], wk2[128:]], axis=1)
    blob[:, WVO : WVO + 16 * O] = np.concatenate(
        [wvo[h, c * 128 : (c + 1) * 128, :] for c in range(2) for h in range(H)],
        axis=1,
    )
    blob[0:8, ID8 : ID8 + 8] = np.eye(8, dtype=f16)
    blob[:, CADJ : CADJ + 2] = cls_adj.reshape(2, 128).T
    blob[0, CADJT : CADJT + D] = cls_adj
    blob[0, BQ : BQ + D] = bq.reshape(D) * scale
    blob[0:BPC, BOUT : BOUT + O] = np.tile(bout.reshape(1, O), (BPC, 1))
    return {"blob": blob}


_NC_CACHE = []


def _get_nc():
    if not _NC_CACHE:
        _NC_CACHE.append(build_program())
    return _NC_CACHE[0]


def run(trace=False, **inputs):
    nc = _get_nc()
    shared = host_inputs(**{k: np.asarray(v) for k, v in inputs.items()})
    x16 = np.asarray(inputs["x"]).astype(np.float16).reshape(B, NT, D)
    in_maps = [
        dict(shared, xs=x16[j * BPC : (j + 1) * BPC]) for j in range(8)
    ]
    res = run_bass_kernel_spmd(nc, in_maps, core_ids=list(range(8)), trace=trace)
    out = np.concatenate([r["out"] for r in res.results], axis=0)
    return out, res


def kernel(**inputs):
    return run(trace=False, **inputs)[0]


# revision 9
# speedup vs baseline: 2.1595x; 1.3506x over previous
"""AttentionPool2d kernel for 8 Trainium2 NeuronCores.

Only the CLS-token output of the attention is returned by the reference, so
the N x N attention collapses to single-query attention per (batch, head):

  t'_m  = x_m + pos_emb[1+m]  (1024 tokens);  t_cls = mean_m(t'_m) + cadj
          with cadj = pos_emb[0] - mean(pos_emb[1:])
  q     = t_cls @ (Wq*scale) + bq*scale                     [256]
  w_s   = sum_k Wk[d,h,k] * q[h*32+k]                       [256, 8]
  scores= t' @ w_s   (bk shifts all logits equally -> softmax-invariant)
  attn  = softmax over 1025 tokens (1024 + CLS)
  u[h]  = sum_m attn[h,m] t'_m + attn_cls*(mean t' + cadj)
        = sum_m (attn[h,m]+attn_cls/1024) t'_m + attn_cls*cadj
  out   = sum_h u[h] @ (Wv[:,h,:] @ Wo[h]) + (bo + sum_h bv[h] @ Wo[h])

Wall time in this environment is dominated by host->device transfer over the
axon tunnel (~50-80 MB/s), so everything large is shipped as float16 and all
weights are packed into a single blob tensor.

Sharding: data-parallel over batch, 8 batches per core.
"""

import sys

sys.path.insert(0, "/opt/trn_rl_repo")

from contextlib import ExitStack

import numpy as np

import concourse.bacc as bacc
import concourse.bass as bass  # noqa: F401
import concourse.tile as tile
from concourse import mybir
from concourse.bass_utils import run_bass_kernel_spmd

F32 = mybir.dt.float32
F16 = mybir.dt.float16
AF = mybir.ActivationFunctionType
ALU = mybir.AluOpType

B, D, H, DK, O = 64, 256, 8, 32, 256
NT = 1024          # non-CLS tokens
BPC = B // 8       # batches per core
NI = NT // 128     # token tiles per batch

# blob column offsets (all f16, [128, C])
POS = 0            # posB [128, 2048], token-part layout
WQ = POS + NI * D          # 2048: wq2*scale, two 128-row halves side by side
WK = WQ + 2 * D            # 2560
WVO = WK + 2 * D           # 3072: 16 chunks [128, 256], chunk (c*H+h)
ID8 = WVO + 16 * O         # 7168: eye(8) on rows 0-7
CADJ = ID8 + 8             # 7176: cls_adj column form [128, 2]
CADJT = CADJ + 2           # 7178: cls_adj row form [1, 256] on row 0
BQ = CADJT + D             # 7434: bq*scale row form [1, 256] on row 0
BOUT = BQ + D              # 7690: bout [8, 256] on rows 0-7
C = BOUT + O               # 7946


def build_program():
    nc = bacc.Bacc(
        "TRN2",
        target_bir_lowering=False,
        debug=False,
        enable_asserts=False,
        num_devices=8,
    )
    xs = nc.dram_tensor("xs", [BPC, NT, D], F16, kind="ExternalInput").ap()
    # each core receives 1/8 of the weight blob; an on-device AllGather
    # reassembles it (saves 14 MB of replicated host->device traffic)
    blobs = nc.dram_tensor("blobs", [16, C], F16, kind="ExternalInput").ap()
    out_d = nc.dram_tensor("out", [BPC, O], F32, kind="ExternalOutput").ap()

    xr = xs.rearrange("b (i p) d -> b p i d", p=128)

    with tile.TileContext(nc) as tc, ExitStack() as ctx:
        wpool = ctx.enter_context(tc.tile_pool(name="weights", bufs=1))
        xpool = ctx.enter_context(tc.tile_pool(name="x", bufs=3))
        tpool = ctx.enter_context(tc.tile_pool(name="xT", bufs=3))
        spool = ctx.enter_context(tc.tile_pool(name="smalls", bufs=4))
        epool = ctx.enter_context(tc.tile_pool(name="esb", bufs=2))
        etpool = ctx.enter_context(tc.tile_pool(name="eT", bufs=2))
        scpool = ctx.enter_context(tc.tile_pool(name="scsb", bufs=2))
        # PSUM: 8 banks total -> 2 + 2 + 1 + 1 + 1 = 7
        q_ps = ctx.enter_context(tc.tile_pool(name="qps", bufs=2, space="PSUM"))
        sc_ps = ctx.enter_context(tc.tile_pool(name="scps", bufs=2, space="PSUM"))
        uT_ps = ctx.enter_context(tc.tile_pool(name="utps", bufs=1, space="PSUM"))
        tr_ps = ctx.enter_context(tc.tile_pool(name="trps", bufs=1, space="PSUM"))

        dram = ctx.enter_context(tc.tile_pool(name="dram", bufs=1, space="DRAM"))
        ag_in = dram.tile([16, C], F16, tag="agin")
        ag_out = dram.tile([128, C], F16, tag="agout")
        nc.gpsimd.dma_start(ag_in[:], blobs)
        nc.gpsimd.collective_compute(
            "AllGather",
            ALU.bypass,
            replica_groups=[list(range(8))],
            ins=[ag_in.opt()],
            outs=[ag_out.opt()],
        )
        blob_s = wpool.tile([128, C], F16, tag="blob")
        nc.sync.dma_start(blob_s[:], ag_out[:])
        cadj32 = wpool.tile([128, 2], F32, tag="cadj32")
        nc.vector.tensor_copy(cadj32[:], blob_s[:, CADJ : CADJ + 2])
        ones16 = wpool.tile([1, 128], F16, tag="ones16")
        nc.vector.memset(ones16[:], 1.0)
        # posT[d, c, m]: transposed positional embedding, from 16 SBUF xbar
        # transposes of posB's [128, 128] sub-tiles (one-time cost)
        posT = wpool.tile([128, 2, NT], F16, tag="posT")
        for i in range(NI):
            for c in range(2):
                nc.sync.dma_start(
                    posT[:, c, i * 128 : (i + 1) * 128],
                    blob_s[:, POS + i * D + c * 128 : POS + i * D + (c + 1) * 128],
                    transpose=True,
                )
        uT_all = wpool.tile([128, 128], F16, tag="uTall")  # (c,b,h) cols

        for b in range(BPC):
            # 1. load x[b] in both layouts: token-major xB and d-major xT
            xB = xpool.tile([128, NI * D], F16, tag="xB")
            nc.sync.dma_start(
                xB[:].rearrange("p (i d) -> p i d", d=D), xr[b]
            )
            xT = tpool.tile([128, 2, NT], F16, tag="xT")
            for c in range(2):
                nc.sync.dma_start(
                    xT[:, c], xs[b][:, c * 128 : (c + 1) * 128], transpose=True
                )
            # 2. t' = x + pos, in place, both layouts
            nc.vector.tensor_tensor(xT[:, 0], xT[:, 0], posT[:, 0], op=ALU.add)
            nc.gpsimd.tensor_tensor(xT[:, 1], xT[:, 1], posT[:, 1], op=ALU.add)
            nc.vector.tensor_tensor(
                xB[:, 0:NT], xB[:, 0:NT], blob_s[:, POS : POS + NT], op=ALU.add
            )
            nc.gpsimd.tensor_tensor(
                xB[:, NT : 2 * NT],
                xB[:, NT : 2 * NT],
                blob_s[:, POS + NT : POS + 2 * NT],
                op=ALU.add,
            )
            # 3. CLS token: mean over tokens + cls_adj
            sums = spool.tile([128, 2], F32, tag="sums")
            t_cls = spool.tile([128, 2], F16, tag="tcls")
            for c in range(2):
                nc.vector.reduce_sum(
                    out=sums[:, c : c + 1], in_=xT[:, c], axis=mybir.AxisListType.X
                )
                nc.vector.tensor_scalar(
                    out=t_cls[:, c : c + 1],
                    in0=sums[:, c : c + 1],
                    scalar1=1.0 / NT,
                    scalar2=cadj32[:, c : c + 1],
                    op0=ALU.mult,
                    op1=ALU.add,
                )
            # 4. q = t_cls @ Wq*scale + bq*scale, broadcast to 128 partitions
            qp = q_ps.tile([1, D], F32, tag="q", name=f"q_{b}")
            for c in range(2):
                nc.tensor.matmul(
                    qp[:],
                    t_cls[:, c : c + 1],
                    blob_s[:, WQ + c * D : WQ + (c + 1) * D],
                    start=(c == 0),
                    stop=(c == 1),
                )
            q_sb = spool.tile([1, D], F16, tag="qsb")
            nc.vector.tensor_tensor(
                q_sb[:], qp[:], blob_s[0:1, BQ : BQ + D], op=ALU.add
            )
            qbc = q_ps.tile([128, D], F32, tag="q", name=f"qbc_{b}")
            nc.tensor.matmul(qbc[:], ones16[:], q_sb[:], start=True, stop=True)
            # 5. w_s[d, h] = sum_k Wk[d, h*32+k] q[h*32+k]
            w_s = spool.tile([128, 2 * H], F16, tag="ws")
            for c in range(2):
                wtmp = spool.tile([128, D], F16, tag="wtmp")
                nc.vector.tensor_tensor(
                    wtmp[:], blob_s[:, WK + c * D : WK + (c + 1) * D], qbc[:],
                    op=ALU.mult,
                )
                with nc.allow_low_precision(reason="w_s stored f16 for the PE"):
                    nc.vector.reduce_sum(
                        out=w_s[:, c * H : (c + 1) * H],
                        in_=wtmp[:].rearrange("p (h k) -> p h k", k=DK),
                        axis=mybir.AxisListType.X,
                    )
            # 6. scores[h, m] = sum_d w_s[d, h] t'[d, m]
            scsb = scpool.tile([H, NT + 32], F32, tag="scsb")
            for lo in (0, 512):
                ps = sc_ps.tile([H, 512], F32, tag="sc", name=f"sc_{b}_{lo}")
                for c in range(2):
                    nc.tensor.matmul(
                        ps[:],
                        w_s[:, c * H : (c + 1) * H],
                        xT[:, c, lo : lo + 512],
                        start=(c == 0),
                        stop=(c == 1),
                    )
                cp = nc.scalar.copy if lo == 0 else nc.vector.tensor_copy
                cp(scsb[:, lo : lo + 512], ps[:])
            ps = sc_ps.tile([H, 512], F32, tag="sc", name=f"sc_{b}_cls")
            for c in range(2):
                nc.tensor.matmul(
                    ps[:, 0:1],
                    w_s[:, c * H : (c + 1) * H],
                    t_cls[:, c : c + 1],
                    start=(c == 0),
                    stop=(c == 1),
                )
            nc.vector.tensor_copy(scsb[:, NT : NT + 1], ps[:, 0:1])
            # 7. softmax over 1025 logits; attn in f16 for the xbar transpose
            nmx = spool.tile([H, 1], F32, tag="nmx")
            nc.vector.reduce_max(
                out=nmx[:], in_=scsb[:, 0 : NT + 1], axis=mybir.AxisListType.X,
                negate=True,
            )
            e_sb = epool.tile([32, NT + 32], F16, tag="esb")
            nc.gpsimd.memset(e_sb[0:32, 0:NT], 0.0)
            zs = spool.tile([H, 1], F32, tag="zs")
            nc.scalar.activation(
                e_sb[0:H, 0 : NT + 1],
                scsb[:, 0 : NT + 1],
                AF.Exp,
                bias=nmx[:],
                scale=1.0,
                accum_out=zs[:],
            )
            rz = spool.tile([H, 1], F32, tag="rz")
            nc.vector.reciprocal(rz[:], zs[:])
            nc.vector.tensor_scalar(
                out=e_sb[0:H, 0 : NT + 1], in0=e_sb[0:H, 0 : NT + 1],
                scalar1=rz[:], scalar2=None, op0=ALU.mult,
            )
            # fold the CLS self-attention back onto the token weights:
            # a'_m = a_m + a_cls/1024  (u += a_cls * mean t')
            acl = spool.tile([H, 1], F32, tag="acl")
            nc.vector.tensor_scalar(
                out=acl[:], in0=e_sb[0:H, NT : NT + 1],
                scalar1=1.0 / NT, scalar2=None, op0=ALU.mult,
            )
            nc.vector.tensor_scalar(
                out=e_sb[0:H, 0:NT], in0=e_sb[0:H, 0:NT],
                scalar1=acl[:], scalar2=None, op0=ALU.add,
            )
            # 8. uT[c][d, h] = sum_m t'[m, d] a'[h, m] + cadj[d] a_cls[h]
            eT = etpool.tile([128, NI, 32], F16, tag="eT")
            nc.sync.dma_start(eT[:], e_sb[:, 0:NT], transpose=True)
            uT = [
                uT_ps.tile([128, H], F32, tag=f"uT{c}", name=f"uT{c}_{b}")
                for c in range(2)
            ]
            for i in range(NI):
                for c in range(2):
                    nc.tensor.matmul(
                        uT[c][:],
                        xB[:, i * D + c * 128 : i * D + (c + 1) * 128],
                        eT[:, i, 0:H],
                        start=(i == 0),
                        stop=False,
                        skip_group_check=True,
                    )
            ecr = tr_ps.tile([1, H], F16, tag="tr", name=f"ecr_{b}")
            nc.tensor.transpose(
                ecr[:], e_sb[0:H, NT : NT + 1], blob_s[0:H, ID8 : ID8 + 8]
            )
            ecs = spool.tile([1, H], F16, tag="ecs")
            nc.vector.tensor_copy(ecs[:], ecr[:])
            for c in range(2):
                nc.tensor.matmul(
                    uT[c][:],
                    blob_s[0:1, CADJT + c * 128 : CADJT + (c + 1) * 128],
                    ecs[:],
                    start=False,
                    stop=True,
                    skip_group_check=True,
                )
                nc.vector.tensor_copy(
                    uT_all[:, c * 64 + b * H : c * 64 + (b + 1) * H], uT[c][:]
                )
        # 9. out[b, o] = sum_{c,h} uT_all[:, c,b,h].T @ Wvo[c,h] + bout
        uv = uT_all[:].rearrange("p (c b h) -> p c b h", c=2, b=BPC)
        o_ps = sc_ps.tile([BPC, O], F32, tag="sc", name="o_ps")
        for c in range(2):
            for h in range(H):
                nc.tensor.matmul(
                    o_ps[:],
                    uv[:, c, :, h],
                    blob_s[:, WVO + (c * H + h) * O : WVO + (c * H + h + 1) * O],
                    start=(c == 0 and h == 0),
                    stop=(c == 1 and h == H - 1),
                )
        o_sb = spool.tile([BPC, O], F32, tag="osb")
        nc.vector.tensor_tensor(
            o_sb[:], o_ps[:], blob_s[0:BPC, BOUT : BOUT + O], op=ALU.add
        )
        nc.sync.dma_start(out_d, o_sb[:])
    nc.compile()
    return nc


def host_inputs(x, pos_emb, Wq, bq, Wk, bk, Wv, bv, Wo, bo):
    """Host-side weight preprocessing shared by all cores (packed f16 blob)."""
    f16 = np.float16
    scale = np.float32(1.0 / np.sqrt(DK))
    pos_rest = pos_emb[1:]
    wq2 = Wq.reshape(D, D) * scale
    wk2 = Wk.reshape(D, H * DK)
    wvo = np.einsum("dhk,hko->hdo", Wv, Wo)
    bout = bo + np.einsum("hk,hko->o", bv, Wo)
    cls_adj = pos_emb[0] - pos_rest.mean(0)

    blob = np.zeros((128, C), f16)
    blob[:, POS : POS + NI * D] = (
        pos_rest.reshape(NI, 128, D).transpose(1, 0, 2).reshape(128, NI * D)
    )
    blob[:, WQ : WQ + 2 * D] = np.concatenate([wq2[:128], wq2[128:]], axis=1)
    blob[:, WK : WK + 2 * D] = np.concatenate([wk2[:128], wk2[128:]], axis=1)
    blob[:, WVO : WVO + 16 * O] = np.concatenate(
        [wvo[h, c * 128 : (c + 1) * 128, :] for c in range(2) for h in range(H)],
        axis=1,
    )
    blob[0:8, ID8 : ID8 + 8] = np.eye(8, dtype=f16)
    blob[:, CADJ : CADJ + 2] = cls_adj.reshape(2, 128).T
    blob[0, CADJT : CADJT + D] = cls_adj
    blob[0, BQ : BQ + D] = bq.reshape(D) * scale
    blob[0:BPC, BOUT : BOUT + O] = np.tile(bout.reshape(1, O), (BPC, 1))
    return blob


_NC_CACHE = []


def _get_nc():
    if not _NC_CACHE:
        _NC_CACHE.append(build_program())
    return _NC_CACHE[0]


def run(trace=False, **inputs):
    nc = _get_nc()
    blob = host_inputs(**{k: np.asarray(v) for k, v in inputs.items()})
    x16 = np.asarray(inputs["x"]).astype(np.float16).reshape(B, NT, D)
    in_maps = [
        {"xs": x16[j * BPC : (j + 1) * BPC], "blobs": blob[16 * j : 16 * (j + 1)]}
        for j in range(8)
    ]
    res = run_bass_kernel_spmd(nc, in_maps, core_ids=list(range(8)), trace=trace)
    out = np.concatenate([r["out"] for r in res.results], axis=0)
    return out, res


def kernel(**inputs):
    return run(trace=False, **inputs)[0]
